# revision 1
# baseline (speedup 1.0000x reference)
"""GridSample (bilinear, zeros padding, align_corners=False, snap-to-ceil
quirk) for Trainium2 via Bass. Batch dim (8) sharded across 8 NeuronCores.

Inputs:  inputs [8,256,256,64] f32, grid [8,256,256,2] f32 in [-1,1)
Output:  [8,256,256,64] f32

Per-core strategy (SWDGE dma_gather):
  - Host packs the per-image feature map into an fp16 table of 2x2-pixel
    blocks: row y*128+i = [img[y,2i], img[y+1,2i], img[y,2i+1],
    img[y+1,2i+1]] (512B). A 768B gather window starting at row
    idx = clamp(y1,0,255)*128 + clamp(floor(x1/2),0,127) <= 32767 (int16)
    contains all 4 bilinear taps for any (x1, y1) in its 6 slot-halves
    [T0 B0 T1 B1 | T0' B0'].
  - Device computes the reference's coordinate transform f32 op-for-op,
    then folds the x-parity (q = x1 - 2*i) and the y-clamp into 6 per-slot
    fp16 weights so the weighted sum over the gathered 64-channel halves
    reproduces wa*A + wb*B + wc*C + wd*D (taps quantized to fp16, output
    written as bf16 and upcast on host).
  - One 1024-descriptor InstDMAGatherAnt per tile (SWDGE Q7 path,
    ~8.4ns/descriptor measured) instead of per-pixel indirect DMAs, which
    run ~90x slower. HW reads gather indices from partitions 16..31 in a
    16-way wrapped layout (differs from the concourse simulator).
"""

import os
import sys

import numpy as np

for _p in ("/opt/trn_rl_repo",):
    if _p not in sys.path and os.path.isdir(_p):
        sys.path.insert(0, _p)

from concourse import bass, mybir
from concourse.bass_utils import run_bass_kernel_spmd
from concourse.library_config import mlp
from concourse.tile import TileContext

# ---------------------------------------------------------------------------
# The walrus build in this container rejects instructions carrying more than
# one sync wait ("Too many sync wait commands", CoreV3GenImpl setupSyncWait).
# TileContext emits multi-wait instructions (e.g. the tail Drain), so split
# every excess wait into its own single-wait EventSemaphore right before the
# owning instruction — semantically identical (engine executes in order).
# ---------------------------------------------------------------------------
import json as _json

from concourse import bass_utils as _bass_utils
from concourse import bass2jax as _bass2jax

_orig_compile_bir_kernel = _bass_utils.compile_bir_kernel


def _split_multiwaits_json(bir_bytes):
    m = _json.loads(bir_bytes)
    changed = False
    for fn in m.get("functions", []):
        for bb in fn.get("blocks", []):
            insts = bb.get("instructions", [])
            out = []
            for ins in insts:
                si = ins.get("sync_info") or {}
                ow = si.get("on_wait") or []
                if len(ow) > 1:
                    changed = True
                    for j, w in enumerate(ow[:-1]):
                        out.append({
                            "debug": ins.get("debug", 0),
                            "engine": ins["engine"],
                            "ins": [],
                            "outs": [],
                            "name": f"{ins['name']}-ws{j}",
                            "opcode": "EventSemaphore",
                            "sync_info": {"on_update": [], "on_wait": [w]},
                        })
                    si["on_wait"] = [ow[-1]]
                out.append(ins)
            bb["instructions"] = out
    if not changed:
        return bir_bytes
    return _json.dumps(m).encode()


def _patched_compile_bir_kernel(bir_json, tmpdir, neff_name="file.neff"):
    return _orig_compile_bir_kernel(
        _split_multiwaits_json(bir_json), tmpdir, neff_name=neff_name)


_bass_utils.compile_bir_kernel = _patched_compile_bir_kernel
_bass2jax.compile_bir_kernel = _patched_compile_bir_kernel

F32 = mybir.dt.float32
F16 = mybir.dt.float16
I16 = mybir.dt.int16
BF16 = mybir.dt.bfloat16

N, H, W, C = 8, 256, 256, 64
NPX = H * W              # 65536 pixels per image
TROWS = H * (W // 2)     # 32768 table rows (2x2 px blocks), +1 pad row
ESTEP = 2 * 2 * C        # 256 f16 elems per table row (512B)
ELEM = 3 * ESTEP // 2    # 384 f16 elems per gather window (768B: the 6
                         # needed slot-halves [T0 B0 T1 B1 | T0' B0'])
NT = 64                  # gather tiles per core (1024 descriptors each:
                         # >=2048 descs per instruction wedges the device —
                         # SWDGE ring carveout is 16KB = 1024 x 16B descs)
TPX = NPX // NT          # 1024 pixels per tile
MCOL = TPX // 128        # 8 pixels per partition per tile
JCOL = NPX // 128        # 512 chain columns per partition (j = t*MCOL + c)
MAGIC = 12582912.0       # 1.5*2^23: x+MAGIC lands in [2^23,2^24) (ulp=1) so
                         # x + MAGIC - MAGIC == rne(x) exactly for |x|<=2^22

_CACHE = {}


def _build_program():
    nc = bass.Bass()
    table = nc.declare_dram_parameter("table", [TROWS + 1, ESTEP], F16,
                                      isOutput=False)
    grid_d = nc.declare_dram_parameter("grid", [NPX, 2], F32, isOutput=False)
    out_d = nc.declare_dram_parameter("out", [NPX, C], BF16, isOutput=True)

    # chain layout: [p, j] with j = t*MCOL + c  <->  pixel t*TPX + p*MCOL + c
    gv = grid_d[:].rearrange("(t p c) k -> p t (c k)", t=NT, p=128)
    ov = out_d[:].rearrange("(t p c) ch -> t p (c ch)", t=NT, p=128)
    win = bass.AP(table, 0, [[ESTEP, TROWS], [1, ELEM]])

    with TileContext(nc) as tc:
        with (
            tc.tile_pool(name="const", bufs=1) as cpool,
            tc.tile_pool(name="gat", bufs=4) as gpool,
            tc.tile_pool(name="acc", bufs=4) as apool,
            tc.tile_pool(name="out", bufs=4) as opool,
        ):
            X = cpool.tile([128, NT * TPX // 16], I16)   # wrapped gather idxs
            w16 = [cpool.tile([128, JCOL], F16, name=f"w16_{h}")
                   for h in range(6)]                     # h = 0,1,2,3,4,6

            with tc.tile_pool(name="chain", bufs=1) as kpool:
                sb_grid = kpool.tile([128, JCOL * 2], F32)
                nc.sync.dma_start(
                    out=sb_grid[:].rearrange("p (t ck) -> p t ck", t=NT),
                    in_=gv)

                # --- coordinate transform, replicating reference f32 ops ---
                def coord_chain(g_view, tag, eng):
                    """Returns (x1, um, fm): x1 = floor(snapped unnormalized
                    coord); um = (x1+1-x)*[x1>=0]; fm = (x-x1)*[x1<=254]."""
                    ts, tt = eng.tensor_scalar, eng.tensor_tensor
                    P = [128, g_view.shape[-1]]
                    x = kpool.tile(P, F32, tag=f"x{tag}")
                    r = kpool.tile(P, F32, tag=f"r{tag}")
                    m = kpool.tile(P, F32, tag=f"m{tag}")
                    d = kpool.tile(P, F32, tag=f"d{tag}")
                    x1 = kpool.tile(P, F32, tag=f"x1{tag}")
                    um = kpool.tile(P, F32, tag=f"um{tag}")
                    fm = kpool.tile(P, F32, tag=f"fm{tag}")
                    # x = ((g + 1) * 256 - 1) * 0.5  (same rounding seq as ref)
                    ts(out=x[:], in0=g_view, scalar1=1.0, scalar2=None,
                       op0=mybir.AluOpType.add)
                    ts(out=x[:], in0=x[:], scalar1=256.0, scalar2=-1.0,
                       op0=mybir.AluOpType.mult, op1=mybir.AluOpType.add)
                    ts(out=x[:], in0=x[:], scalar1=0.5, scalar2=None,
                       op0=mybir.AluOpType.mult)
                    # snap-to-ceil: cc = ceil(x); if cc - x < 1e-5: x = cc
                    ts(out=r[:], in0=x[:], scalar1=MAGIC, scalar2=-MAGIC,
                       op0=mybir.AluOpType.add, op1=mybir.AluOpType.add)
                    tt(out=m[:], in0=r[:], in1=x[:], op=mybir.AluOpType.is_lt)
                    tt(out=r[:], in0=r[:], in1=m[:], op=mybir.AluOpType.add)
                    tt(out=d[:], in0=r[:], in1=x[:],
                       op=mybir.AluOpType.subtract)
                    ts(out=m[:], in0=d[:], scalar1=1e-5, scalar2=None,
                       op0=mybir.AluOpType.is_lt)
                    tt(out=d[:], in0=d[:], in1=m[:], op=mybir.AluOpType.mult)
                    tt(out=x[:], in0=x[:], in1=d[:], op=mybir.AluOpType.add)
                    # x1 = floor(x)
                    ts(out=r[:], in0=x[:], scalar1=MAGIC, scalar2=-MAGIC,
                       op0=mybir.AluOpType.add, op1=mybir.AluOpType.add)
                    tt(out=m[:], in0=r[:], in1=x[:], op=mybir.AluOpType.is_gt)
                    tt(out=x1[:], in0=r[:], in1=m[:],
                       op=mybir.AluOpType.subtract)
                    # fm = (x - x1) * [x1 <= 254]
                    tt(out=fm[:], in0=x[:], in1=x1[:],
                       op=mybir.AluOpType.subtract)
                    ts(out=m[:], in0=x1[:], scalar1=254.0, scalar2=None,
                       op0=mybir.AluOpType.is_le)
                    tt(out=fm[:], in0=fm[:], in1=m[:], op=mybir.AluOpType.mult)
                    # um = ((x1 + 1) - x) * [x1 >= 0]
                    ts(out=r[:], in0=x1[:], scalar1=1.0, scalar2=None,
                       op0=mybir.AluOpType.add)
                    tt(out=um[:], in0=r[:], in1=x[:],
                       op=mybir.AluOpType.subtract)
                    ts(out=m[:], in0=x1[:], scalar1=0.0, scalar2=None,
                       op0=mybir.AluOpType.is_ge)
                    tt(out=um[:], in0=um[:], in1=m[:], op=mybir.AluOpType.mult)
                    return x1, um, fm

                gx = sb_grid[:, 0::2]
                gy = sb_grid[:, 1::2]
                x1, umx, fmx = coord_chain(gx, "x", nc.vector)
                y1, umy, fmy = coord_chain(gy, "y", nc.vector)

                P = [128, JCOL]
                tt = nc.vector.tensor_tensor
                ts = nc.vector.tensor_scalar
                A = mybir.AluOpType

                # x pair index i = clamp(floor(x1/2), 0) and parity q
                i0 = kpool.tile(P, F32)
                q = kpool.tile(P, F32)
                tmp = kpool.tile(P, F32)
                ts(out=i0[:], in0=x1[:], scalar1=0.5, scalar2=-0.25,
                   op0=A.mult, op1=A.add)
                ts(out=i0[:], in0=i0[:], scalar1=MAGIC, scalar2=-MAGIC,
                   op0=A.add, op1=A.add)           # rne -> floor(x1/2)
                ts(out=i0[:], in0=i0[:], scalar1=0.0, scalar2=None,
                   op0=A.max)
                ts(out=tmp[:], in0=i0[:], scalar1=2.0, scalar2=None,
                   op0=A.mult)
                tt(out=q[:], in0=x1[:], in1=tmp[:], op=A.subtract)  # -1/0/1

                # gather index = clamp(y1,0)*128 + i0, as int16
                idxf = kpool.tile(P, F32)
                ts(out=idxf[:], in0=y1[:], scalar1=0.0, scalar2=None,
                   op0=A.max)
                ts(out=idxf[:], in0=idxf[:], scalar1=128.0, scalar2=None,
                   op0=A.mult)
                tt(out=idxf[:], in0=idxf[:], in1=i0[:], op=A.add)
                idx16 = kpool.tile(P, I16)
                nc.vector.tensor_copy(out=idx16[:], in_=idxf[:])

                # scatter idx16 [p, j] into the ucode's wrapped idx layout.
                # Measured on HW (probe_gather): slot i of a gather reads
                # X[16 + i%16, i//16] — partitions 16..31, NOT 0..15 as the
                # concourse simulator models. Slot i lands at dst partition
                # i%128 / column i//128 = pixel (p, c).
                # Two stages so the partition-crossing DMAs stay contiguous
                # (the direct scatter would be 2-byte descriptors): first fold
                # partition blocks s8*16..s8*16+16 into columns of S, then a
                # single DVE copy permutes the free dim into wrapped order.
                S = kpool.tile([128, NT * TPX // 16], I16)
                nc.vector.memset(S[:], 0)
                for s8 in range(8):
                    nc.sync.dma_start(
                        out=S[16:32, s8 * JCOL:(s8 + 1) * JCOL],
                        in_=idx16[s8 * 16:(s8 + 1) * 16, :])
                nc.vector.tensor_copy(
                    out=X[:, :].rearrange("p (t m e) -> p t m e",
                                          t=NT, m=MCOL),
                    in_=S[:, :].rearrange("p (e t m) -> p t m e",
                                          e=8, t=NT))

                wa = kpool.tile(P, F32)
                wb = kpool.tile(P, F32)
                wc = kpool.tile(P, F32)
                wd = kpool.tile(P, F32)
                tt(out=wa[:], in0=umx[:], in1=umy[:], op=A.mult)
                tt(out=wb[:], in0=umx[:], in1=fmy[:], op=A.mult)
                tt(out=wc[:], in0=fmx[:], in1=umy[:], op=A.mult)
                tt(out=wd[:], in0=fmx[:], in1=fmy[:], op=A.mult)

                # fold y-clamp into row weights: when y1 < 0 the gathered
                # block's TOP row is img row 0 = the bilinear BOTTOM taps.
                m1y = kpool.tile(P, F32)
                va = kpool.tile(P, F32)
                vb = kpool.tile(P, F32)
                vc = kpool.tile(P, F32)
                vd = kpool.tile(P, F32)
                ts(out=m1y[:], in0=y1[:], scalar1=0.0, scalar2=None,
                   op0=A.is_lt)
                tt(out=tmp[:], in0=m1y[:], in1=wb[:], op=A.mult)
                tt(out=va[:], in0=wa[:], in1=tmp[:], op=A.add)
                tt(out=vb[:], in0=wb[:], in1=tmp[:], op=A.subtract)
                tt(out=tmp[:], in0=m1y[:], in1=wd[:], op=A.mult)
                tt(out=vc[:], in0=wc[:], in1=tmp[:], op=A.add)
                tt(out=vd[:], in0=wd[:], in1=tmp[:], op=A.subtract)

                # parity indicators and the 6 per-slot-half weights
                eqm1 = kpool.tile(P, F32)
                eq0 = kpool.tile(P, F32)
                eq1 = kpool.tile(P, F32)
                ts(out=eqm1[:], in0=q[:], scalar1=-1.0, scalar2=None,
                   op0=A.is_equal)
                ts(out=eq0[:], in0=q[:], scalar1=0.0, scalar2=None,
                   op0=A.is_equal)
                ts(out=eq1[:], in0=q[:], scalar1=1.0, scalar2=None,
                   op0=A.is_equal)

                wt = kpool.tile(P, F32, tag="wt")

                def slotw(dst16, m_a, e_a, m_b, e_b):
                    """dst16 = f16(m_a*e_a + (m_b*e_b if given))"""
                    tt(out=wt[:], in0=m_a[:], in1=e_a[:], op=A.mult)
                    if m_b is not None:
                        tt(out=tmp[:], in0=m_b[:], in1=e_b[:], op=A.mult)
                        tt(out=wt[:], in0=wt[:], in1=tmp[:], op=A.add)
                    nc.vector.tensor_copy(out=dst16[:], in_=wt[:])

                slotw(w16[0], va, eq0, vc, eqm1)   # h0 = T0
                slotw(w16[1], vb, eq0, vd, eqm1)   # h1 = B0
                slotw(w16[2], va, eq1, vc, eq0)    # h2 = T1
                slotw(w16[3], vb, eq1, vd, eq0)    # h3 = B1
                slotw(w16[4], vc, eq1, None, None)  # h4 = T0'
                slotw(w16[5], vd, eq1, None, None)  # h5 = B0'

            # --- gather + weighted sum, two gather tiles per sum ---
            # (halves DVE/out-DMA instruction count: less dispatch traffic
            # competing with the gather ucode)
            ov2 = out_d[:].rearrange("(u v p c) ch -> u p v (c ch)",
                                     u=NT // 2, v=2, p=128)
            nc.gpsimd.load_library(mlp)
            nreg = nc.gpsimd.to_reg(TPX)   # one shared count register
            M2 = 2 * MCOL
            for t in range(0, NT, 2):
                G = gpool.tile([128, M2, ELEM], F16)
                for v in range(2):
                    nc.gpsimd.dma_gather(
                        out_ap=G[:, v * MCOL:(v + 1) * MCOL, :],
                        in_ap=win,
                        idxs_ap=X[:, (t + v) * (TPX // 16):
                                  (t + v + 1) * (TPX // 16)],
                        num_idxs=TPX,
                        num_idxs_reg=nreg,
                        elem_size=ELEM,
                        elem_step=ESTEP,
                    )
                sl = slice(t * MCOL, (t + 2) * MCOL)
                att = nc.any.tensor_tensor
                t1 = apool.tile([128, M2, C], F16, tag="t1")
                t2 = apool.tile([128, M2, C], F16, tag="t2")
                O = opool.tile([128, M2 * C], BF16)
                Ov = O[:].rearrange("p (m ch) -> p m ch", m=M2)
                # gathered halves: [T0 T1 B0 B1 | T0' T1' B0' B1'] x 64ch
                HSLOT = (0, 1, 2, 3, 4, 5)

                def half(h):
                    return G[:, :, h * C:(h + 1) * C]

                def wbc(k):
                    return w16[k][:, sl].to_broadcast([128, M2, C])

                att(out=t1[:], in0=half(HSLOT[0]), in1=wbc(0), op=A.mult)
                att(out=t2[:], in0=half(HSLOT[1]), in1=wbc(1), op=A.mult)
                att(out=t1[:], in0=t1[:], in1=t2[:], op=A.add)
                att(out=t2[:], in0=half(HSLOT[2]), in1=wbc(2), op=A.mult)
                att(out=t1[:], in0=t1[:], in1=t2[:], op=A.add)
                att(out=t2[:], in0=half(HSLOT[3]), in1=wbc(3), op=A.mult)
                att(out=t1[:], in0=t1[:], in1=t2[:], op=A.add)
                att(out=t2[:], in0=half(HSLOT[4]), in1=wbc(4), op=A.mult)
                att(out=t1[:], in0=t1[:], in1=t2[:], op=A.add)
                att(out=t2[:], in0=half(HSLOT[5]), in1=wbc(5), op=A.mult)
                att(out=Ov, in0=t1[:], in1=t2[:], op=A.add)
                nc.sync.dma_start(out=ov2[t // 2], in_=O[:])

    mybir.codegen_inst_isa_subclasses(nc)
    return nc


def _get_program():
    if "nc" not in _CACHE:
        _CACHE["nc"] = _build_program()
    return _CACHE["nc"]


def _make_in_maps(inputs, grid):
    in_maps = []
    for i in range(N):
        img = np.ascontiguousarray(inputs[i]).astype(np.float16)
        imgp = np.zeros((H + 1, W, C), dtype=np.float16)
        imgp[:H] = img
        T = imgp[0:H].reshape(H, W // 2, 2, C)
        B = imgp[1:H + 1].reshape(H, W // 2, 2, C)
        tab = np.zeros((TROWS + 1, ESTEP), dtype=np.float16)
        tab[:TROWS] = np.concatenate(
            [T[:, :, 0:1], B[:, :, 0:1], T[:, :, 1:2], B[:, :, 1:2]],
            axis=2).reshape(TROWS, ESTEP)
        g = np.ascontiguousarray(grid[i], dtype=np.float32).reshape(NPX, 2)
        in_maps.append({"table": tab, "grid": g})
    return in_maps


def run(inputs, grid, trace=False, **kw):
    nc = _get_program()
    in_maps = _make_in_maps(inputs, grid)
    res = run_bass_kernel_spmd(nc, in_maps, list(range(N)), trace=trace, **kw)
    out = np.empty((N, H, W, C), dtype=np.float32)
    for i in range(N):
        out[i] = np.asarray(res.results[i]["out"]).astype(np.float32).reshape(H, W, C)
    return out, res


def kernel(inputs, grid):
    out, _ = run(inputs, grid, trace=False)
    return out

